# revision 10
# baseline (speedup 1.0000x reference)
"""GNN message-passing encoder (GAT-style) on 8 Trainium2 NeuronCores — v4.

Self-contained: hardcodes the problem shapes (N=100000, E=1600000, HN=64, L=3).

Design (edge-major, bf16, scatterless):
- Core c owns dst nodes [c*NL, (c+1)*NL). Node space re-keyed into G=4 chunks:
  chunk s = slice s of every core (SP=3200 padded rows per core-slice), so each
  chunk's table is completed by ONE sub-AllGather -> gather of chunk g overlaps
  the AllGather of chunk g+1.
- Per layer: table build emits node-major bf16 rows [a_s | hl] per slice,
  4 sub-AllGathers; a_d table kept local node-major (127-node subchunks,
  row 127 = u_l so the ea*u edge term rides the same expand matmul).
- Edges sorted by (chunk, dst-subchunk, dst); tiles of 128 edges never span a
  (chunk, subchunk); blocks of 32 tiles never span a chunk. Per block:
  dma_gather 4096 src rows (256B bf16); one-hot masks od (dst-major, via K=1
  broadcast matmul + is_eq) and odT (edge-major, via is_eq against iota);
  expand matmul a_d+ea*u per tile; alpha = psx + a_s; LeakyReLU; Exp;
  contrib = [ex*hl | ex]; aggregation matmul accumulates [128 dst, 128 ch]
  PSUM across the subchunk's tiles; flushed by DVE add into an SBUF
  accumulator — no scatter, no HBM accum round-trip.
- Finalize: h = num/(den+1e-16) from SBUF, PE-transpose back to channel-major
  for the next layer; final layer DMAs node-major rows out.
"""
import os
import sys

for _p in ("/opt/trn_rl_repo",):
    if _p not in sys.path:
        sys.path.insert(0, _p)

import numpy as np
import ml_dtypes

import concourse.bass as bass
import concourse.mybir as mybir
import concourse.tile as tile
from concourse import bacc
from concourse import library_config as libcfg
from concourse.bass_utils import run_bass_kernel_spmd

F32 = mybir.dt.float32
BF16 = mybir.dt.bfloat16
I16 = mybir.dt.int16
AX = mybir.AluOpType
AF = mybir.ActivationFunctionType
BF16NP = ml_dtypes.bfloat16
FP8 = mybir.dt.float8e4
ODT_NP = ml_dtypes.float8_e4m3


class Cfg:
    def __init__(self, N=100000, E=1600000, BT=32, B=10):
        self.N, self.E, self.C, self.HN, self.IN_N, self.L, self.B = \
            N, E, 8, 64, 3, 3, B
        self.NEG, self.EPS = 0.2, 1e-5
        self.NL = self.N // self.C          # 12500 local dst nodes
        self.G = 4                          # src chunks
        self.S = self.NL // self.G          # 3125 real rows per slice
        self.SP = -(-self.S // 128) * 128   # 3200 padded slice rows
        self.CH = self.C * self.SP          # 25600 chunk rows
        self.RP = self.SP // 128            # staging row-groups per slice
        self.SC = 127                       # dst nodes per subchunk
        self.NSC = -(-self.NL // self.SC)   # 99 subchunks
        self.TE = 128
        self.BT = BT
        self.SUB = min(8, BT)
        self.EB = self.TE * BT              # 4096 edges per block
        assert self.NL == self.G * self.S and self.CH <= 32768


# ------------------------------------------------------------- host prep ----
def _pack_idx16(vals):
    v = np.asarray(vals, dtype=np.int16)
    assert v.size % 16 == 0
    return np.tile(v.reshape(-1, 16).T, (8, 1)).copy()


def prep_edges(cfg, src, dst, ea):
    """Per-core edge partition grouped by (g, sc); returns per-core dict of
    (g, sc) -> (srcidx array, code array, ea array)."""
    per_core = []
    for c in range(cfg.C):
        m = (dst >= c * cfg.NL) & (dst < (c + 1) * cfg.NL)
        s_c, d_c, e_c = src[m], dst[m] - c * cfg.NL, ea[m]
        g = (s_c % cfg.NL) // cfg.S
        sidx = (s_c // cfg.NL) * cfg.SP + (s_c % cfg.S)
        sc = d_c // cfg.SC
        code = d_c % cfg.SC
        order = np.lexsort((d_c, sc, g))
        per_core.append(dict(g=g[order], sidx=sidx[order], sc=sc[order],
                             code=code[order], ea=e_c[order]))
    return per_core


def build_tiles(cfg, per_core):
    """Global tile layout (shared across cores) + per-core packed streams."""
    C, G, NSC, TE, BT = cfg.C, cfg.G, cfg.NSC, cfg.TE, cfg.BT
    # edge counts per (core, g, sc)
    cnt = np.zeros((C, G, NSC), np.int64)
    for c in range(C):
        pc = per_core[c]
        np.add.at(cnt[c], (pc["g"], pc["sc"]), 1)
    T = np.maximum(cnt, 0).max(axis=0)          # [G, NSC] max count
    T = -(-T // TE)                              # tiles per (g, sc)
    # pad each g to a multiple of BT (extend last subchunk's tile count)
    for g in range(G):
        rem = (-T[g].sum()) % BT
        T[g, NSC - 1] += rem
    NBg = [int(T[g].sum()) // BT for g in range(G)]
    TT = int(T.sum())
    NB = sum(NBg)
    assert TT == NB * BT

    # shared per-tile metadata
    tile_g = np.zeros(TT, np.int32)
    tile_sc = np.zeros(TT, np.int32)
    tile_st = np.zeros(TT, bool)
    tile_sp = np.zeros(TT, bool)
    t = 0
    for g in range(G):
        for sc in range(NSC):
            n = int(T[g, sc])
            if n == 0:
                continue
            tile_g[t:t + n] = g
            tile_sc[t:t + n] = sc
            tile_st[t] = True
            tile_sp[t + n - 1] = True
            t += n
    assert t == TT

    # per-core streams
    streams = []
    d127 = np.arange(127)
    for c in range(C):
        pc = per_core[c]
        gidx = np.zeros(TT * TE, np.int16)
        codeF = np.full(TT * TE, -1, np.int32)
        earow = np.zeros(TT * TE, np.float32)
        # group start offsets in the sorted stream
        t0 = 0
        p = 0  # pointer into sorted edge stream
        for g in range(G):
            for sc in range(NSC):
                n_edges = int(cnt[c, g, sc])
                ntiles = int(T[g, sc])
                base = t0 * TE
                sl = slice(p, p + n_edges)
                gidx[base:base + n_edges] = pc["sidx"][sl].astype(np.int16)
                codeF[base:base + n_edges] = pc["code"][sl]
                earow[base:base + n_edges] = pc["ea"][sl]
                p += n_edges
                t0 += ntiles
        assert p == len(pc["g"])
        # od[d, e] one-hot (dst-major) with ea on row 127; odT[p, t, d]
        od = np.zeros((128, TT * TE), ODT_NP)
        od[:127, :] = (codeF[None, :] == d127[:, None]).astype(ODT_NP)
        od[127, :] = earow.astype(ODT_NP)
        odT = (codeF.reshape(TT, TE).T[:, :, None]
               == d127[None, None, :]).astype(ODT_NP)   # [p, TT, 127]
        odTf = np.zeros((128, TT, 128), ODT_NP)
        odTf[:, :, :127] = odT
        streams.append(dict(
            gidx=_pack_idx16(gidx),
            odh=od.reshape(128, NB, cfg.EB).transpose(1, 0, 2).copy(),
            odth=odTf.reshape(128, NB, BT * 128).transpose(1, 0, 2).copy(),
        ))
    meta = dict(NB=NB, NBg=NBg, TT=TT, tile_g=tile_g, tile_sc=tile_sc,
                tile_st=tile_st, tile_sp=tile_sp)
    return meta, streams


def host_prepare(cfg, inputs):
    ei = np.asarray(inputs["edge_index"])
    src, dst = ei[0].astype(np.int64), ei[1].astype(np.int64)
    ea = np.asarray(inputs["edge_attr"], np.float32)[:, 0]
    per_core = prep_edges(cfg, src, dst, ea)
    meta, streams = build_tiles(cfg, per_core)

    HN, L = cfg.HN, cfg.L
    fnw = np.asarray(inputs["fc_node_w"], np.float32)
    fnb = np.asarray(inputs["fc_node_b"], np.float32)
    few = np.asarray(inputs["fc_edge_w"], np.float32)
    feb = np.asarray(inputs["fc_edge_b"], np.float32)
    gam = np.asarray(inputs["bn_gamma"], np.float32)
    bet = np.asarray(inputs["bn_beta"], np.float32)
    fcw = np.asarray(inputs["fc_w"], np.float32)
    fcb = np.asarray(inputs["fc_b"], np.float32)
    aw = np.asarray(inputs["attn_w"], np.float32)
    ab = np.asarray(inputs["attn_b"], np.float32)
    wdst, wsrc, we = aw[:, :HN, :], aw[:, HN:2 * HN, :], aw[:, 2 * HN:, :]
    u = np.stack([few[0] @ we[l] for l in range(L)])          # [L, 64]
    adb = np.stack([ab[l] + feb @ we[l] for l in range(L)])   # [L, 64]
    x = np.asarray(inputs["x"], np.float32)

    shared = dict(
        fnw=fnw, fnb=fnb.reshape(-1, 1), gam=gam.reshape(-1, 1),
        bet=bet.reshape(-1, 1),
        fcw=np.ascontiguousarray(fcw.transpose(1, 0, 2)).astype(BF16NP),
        wsrc=np.ascontiguousarray(wsrc.transpose(1, 0, 2)).astype(BF16NP),
        wdst=np.ascontiguousarray(wdst.transpose(1, 0, 2)).astype(BF16NP),
        fcb=np.ascontiguousarray(fcb.T), adb=np.ascontiguousarray(adb.T),
        urow=u.astype(BF16NP),                                 # [L, 64]
        urow_b=np.broadcast_to(u.astype(BF16NP)[:, None, :],
                               (L, cfg.NSC, HN)).copy(),
        identb=np.eye(128, dtype=BF16NP),
    )
    in_maps = []
    for c in range(cfg.C):
        im = dict(shared)
        im.update(streams[c])
        im["xT"] = np.ascontiguousarray(
            x[c * cfg.NL:(c + 1) * cfg.NL].T).astype(BF16NP)
        in_maps.append(im)
    return in_maps, meta


# --------------------------------------------------------------- builder ----
def split_sync_waits(nc, max_waits=1):
    for f in nc.m.functions:
        for bb in f.blocks:
            old = bb.instructions
            if not any(i.sync_info and i.sync_info.on_wait
                       and len(i.sync_info.on_wait) > max_waits for i in old):
                continue
            new = []
            for ins in old:
                si = ins.sync_info
                if si is not None and si.on_wait and len(si.on_wait) > max_waits:
                    waits = list(si.on_wait)
                    extra, keep = waits[:-max_waits], waits[-max_waits:]
                    for j, w in enumerate(extra):
                        nop = mybir.InstNoOp(name=f"{ins.name}-wc{j}", ins=[], outs=[])
                        nop.engine = ins.engine
                        nop.sync_info = mybir.SyncInfo(on_wait=[w], on_update=[])
                        new.append(nop)
                    si.on_wait = keep
                new.append(ins)
            bb.instructions = new
    return nc


def build_nc(cfg, meta):
    NL, G, S, SP, CH = cfg.NL, cfg.G, cfg.S, cfg.SP, cfg.CH
    SC, NSC, TE, BT, EB = cfg.SC, cfg.NSC, cfg.TE, cfg.BT, cfg.EB
    HN, L, C, N = cfg.HN, cfg.L, cfg.C, cfg.N
    NB, NBg, TT = meta["NB"], meta["NBg"], meta["TT"]
    tile_g, tile_sc = meta["tile_g"], meta["tile_sc"]
    tile_st, tile_sp = meta["tile_st"], meta["tile_sp"]
    RG = [list(range(C))]

    nc = bacc.Bacc(None, target_bir_lowering=False, num_swdge_queues=4)
    din = {}

    def ext(name, shape, dt=F32):
        din[name] = nc.dram_tensor(name, shape, dt, kind="ExternalInput")

    ext("xT", [cfg.IN_N, NL], BF16)
    ext("gidx", [128, TT * TE // 16], I16)
    ext("odh", [NB, 128, EB], FP8)
    ext("odth", [NB, 128, BT * 128], FP8)
    ext("fnw", [cfg.IN_N, HN]); ext("fnb", [HN, 1])
    ext("gam", [HN, 1]); ext("bet", [HN, 1])
    ext("fcw", [HN, L, HN], BF16); ext("fcb", [HN, L])
    ext("wsrc", [HN, L, HN], BF16); ext("wdst", [HN, L, HN], BF16)
    ext("adb", [HN, L])
    ext("urow", [L, HN], BF16)
    ext("urow_b", [L, NSC, HN], BF16)
    ext("identb", [128, 128], BF16)

    out = nc.dram_tensor("out", [NL, HN], F32, kind="ExternalOutput")
    lt = [nc.dram_tensor(f"lt{s}", [SP, 128], BF16) for s in range(G)]
    fsh = [nc.dram_tensor(f"fsh{s}", [C, SP, 128], BF16, addr_space="Shared")
           for s in range(G)]
    bn_loc = nc.dram_tensor("bn_loc", [HN, 2], F32)
    bn_sh = nc.dram_tensor("bn_sh", [HN, 2], F32, addr_space="Shared")

    with tile.TileContext(nc) as tc:
        with tc.tile_pool(name="persist", bufs=1) as pp:
            nc.gpsimd.load_library(libcfg.mlp)
            w = {}
            for nm, shp, dt in (
                ("fnw", [cfg.IN_N, HN], F32), ("fnb", [HN, 1], F32),
                ("gam", [HN, 1], F32), ("bet", [HN, 1], F32),
                ("fcw", [HN, L, HN], BF16), ("fcb", [HN, L], F32),
                ("wsrc", [HN, L, HN], BF16), ("wdst", [HN, L, HN], BF16),
                ("adb", [HN, L], F32), ("urow", [L, HN], BF16),
                ("identb", [128, 128], BF16),
            ):
                w[nm] = pp.tile(shp, dt, tag=nm, name=nm)
                nc.sync.dma_start(out=w[nm][:], in_=din[nm][:])
            ident = w["identb"]
            hTb = pp.tile([HN, NL], BF16, tag="hTb", name="hTb")
            ad_nm = pp.tile([128, NSC, HN], BF16, tag="ad_nm", name="ad_nm")
            accum = pp.tile([128, NSC, 128], F32, tag="accum", name="accum")

            # ---------------- input proj + BN (f32) ----------------
            with tc.tile_pool(name="bn", bufs=2) as bp, \
                 tc.tile_pool(name="bnsq", bufs=1) as bq, \
                 tc.tile_pool(name="bnp", bufs=2, space="PSUM") as bpp:
                xt = bq.tile([cfg.IN_N, NL], BF16, tag="xt")
                nc.sync.dma_start(out=xt[:], in_=din["xT"][:])
                fnwb = bp.tile([cfg.IN_N, HN], BF16, tag="fnwb")
                nc.vector.tensor_copy(fnwb[:], w["fnw"][:])
                for ci in range(-(-NL // 512)):
                    c0 = ci * 512
                    n = min(512, NL - c0)
                    ps = bpp.tile([HN, 512], F32, tag="ps")
                    nc.tensor.matmul(out=ps[:, :n], lhsT=fnwb[:],
                                     rhs=xt[:, c0:c0 + n], start=True, stop=True)
                    nc.vector.tensor_scalar_add(hTb[:, c0:c0 + n], ps[:, :n],
                                                w["fnb"][:])
                st = bp.tile([HN, 2], F32, tag="st")
                stp = bp.tile([HN, cfg.G], F32, tag="stp")
                sqc = bq.tile([HN, S], F32, tag="sqc")
                nc.vector.reduce_sum(st[:, 0:1], hTb[:], axis=mybir.AxisListType.X)
                for j in range(cfg.G):
                    nc.vector.scalar_tensor_tensor(
                        out=sqc[:], in0=hTb[:, j * S:(j + 1) * S], scalar=1.0,
                        in1=hTb[:, j * S:(j + 1) * S], op0=AX.mult, op1=AX.mult,
                        accum_out=stp[:, j:j + 1])
                nc.vector.reduce_sum(st[:, 1:2], stp[:], axis=mybir.AxisListType.X)
                nc.sync.dma_start(out=bn_loc[:], in_=st[:])
                nc.gpsimd.collective_compute("AllReduce", AX.add,
                                             replica_groups=RG,
                                             ins=[bn_loc[:]], outs=[bn_sh[:]])
                sg = bp.tile([HN, 2], F32, tag="sg")
                nc.sync.dma_start(out=sg[:], in_=bn_sh[:])
                mean = bp.tile([HN, 1], F32, tag="mean")
                var = bp.tile([HN, 1], F32, tag="var")
                nc.vector.tensor_scalar_mul(mean[:], sg[:, 0:1], 1.0 / N)
                nc.vector.tensor_scalar_mul(var[:], sg[:, 1:2], 1.0 / N)
                msq = bp.tile([HN, 1], F32, tag="msq")
                nc.vector.tensor_mul(msq[:], mean[:], mean[:])
                nc.vector.tensor_sub(var[:], var[:], msq[:])
                nc.vector.tensor_scalar_add(var[:], var[:], cfg.EPS)
                rs = bp.tile([HN, 1], F32, tag="rs")
                nc.scalar.activation(out=rs[:], in_=var[:], func=AF.Sqrt)
                nc.vector.reciprocal(rs[:], rs[:])
                scale = bp.tile([HN, 1], F32, tag="scale")
                nc.vector.tensor_mul(scale[:], rs[:], w["gam"][:])
                nbias = bp.tile([HN, 1], F32, tag="nbias")
                nc.vector.tensor_mul(nbias[:], mean[:], scale[:])
                nc.vector.scalar_tensor_tensor(out=nbias[:], in0=nbias[:],
                                               scalar=-1.0, in1=w["bet"][:],
                                               op0=AX.mult, op1=AX.add)
                nc.vector.tensor_scalar(out=hTb[:], in0=hTb[:], scalar1=scale[:],
                                        scalar2=nbias[:], op0=AX.mult, op1=AX.add)

            # ---------------- layers ----------------
            for l in range(L):
                # --- table build ---
                with tc.tile_pool(name=f"tb{l}", bufs=2) as tp, \
                     tc.tile_pool(name=f"tq{l}", bufs=1) as tq, \
                     tc.tile_pool(name=f"tp{l}", bufs=2, space="PSUM") as tpp:
                    adT = tq.tile([HN, NL], BF16, tag="adT")
                    nc.vector.memset(ad_nm[:], 0.0)
                    for s in range(G):
                        hls = tp.tile([HN, S], BF16, tag="hls")
                        asl = tp.tile([HN, S], BF16, tag="asl")
                        for ci in range(-(-S // 512)):
                            c0 = ci * 512
                            n = min(512, S - c0)
                            a0 = s * S + c0
                            ph = tpp.tile([HN, 512], F32, tag="ph")
                            nc.tensor.matmul(out=ph[:, :n], lhsT=w["fcw"][:, l, :],
                                             rhs=hTb[:, a0:a0 + n],
                                             start=True, stop=True)
                            nc.vector.tensor_scalar_add(hls[:, c0:c0 + n],
                                                        ph[:, :n],
                                                        w["fcb"][:, l:l + 1])
                            psa = tpp.tile([HN, 512], F32, tag="ph")
                            nc.tensor.matmul(out=psa[:, :n], lhsT=w["wsrc"][:, l, :],
                                             rhs=hls[:, c0:c0 + n],
                                             start=True, stop=True)
                            nc.scalar.copy(out=asl[:, c0:c0 + n], in_=psa[:, :n])
                            psd = tpp.tile([HN, 512], F32, tag="ph")
                            nc.tensor.matmul(out=psd[:, :n], lhsT=w["wdst"][:, l, :],
                                             rhs=hls[:, c0:c0 + n],
                                             start=True, stop=True)
                            nc.vector.tensor_scalar_add(adT[:, a0:a0 + n],
                                                        psd[:, :n],
                                                        w["adb"][:, l:l + 1])
                        # node-major rows for slice s
                        stg = tp.tile([128, cfg.RP, 128], BF16, tag="stg")
                        nc.vector.memset(stg[:], 0.0)
                        for r in range(cfg.RP):
                            c0 = r * 128
                            n = min(128, S - c0)
                            pt1 = tpp.tile([128, HN], BF16, tag="pt")
                            nc.tensor.transpose(out=pt1[:n, :],
                                                in_=asl[:, c0:c0 + n],
                                                identity=ident[:HN, :HN])
                            nc.scalar.copy(out=stg[:n, r, 0:HN], in_=pt1[:n, :])
                            pt2 = tpp.tile([128, HN], BF16, tag="pt")
                            nc.tensor.transpose(out=pt2[:n, :],
                                                in_=hls[:, c0:c0 + n],
                                                identity=ident[:HN, :HN])
                            nc.scalar.copy(out=stg[:n, r, HN:128], in_=pt2[:n, :])
                        nc.sync.dma_start(
                            out=lt[s][:, :].rearrange("(r p) k -> p r k", p=128),
                            in_=stg[:, :, :])
                        nc.gpsimd.collective_compute("AllGather", AX.bypass,
                                                     replica_groups=RG,
                                                     ins=[lt[s][:]],
                                                     outs=[fsh[s][:]])
                    # a_d node-major subtables
                    for sc in range(NSC):
                        c0 = sc * SC
                        n = min(SC, NL - c0)
                        pta = tpp.tile([128, HN], BF16, tag="pt")
                        nc.tensor.transpose(out=pta[:n, :],
                                            in_=adT[:, c0:c0 + n],
                                            identity=ident[:HN, :HN])
                        nc.scalar.copy(out=ad_nm[:n, sc, :], in_=pta[:n, :])
                    nc.sync.dma_start(out=ad_nm[127:128, :, :],
                                      in_=din["urow_b"][l:l + 1, :, :])
                    nc.vector.memset(accum[:], 0.0)

                # --- edge blocks ---
                with tc.tile_pool(name=f"ep{l}", bufs=4) as ep, \
                     tc.tile_pool(name=f"px{l}", bufs=4, space="PSUM") as pxp, \
                     tc.tile_pool(name=f"pa{l}", bufs=4, space="PSUM") as pap:
                    agg_state = [None]

                    def emit_aggs(prev):
                        t_base0, odt0, contrib0 = prev
                        for t2 in range(BT):
                            t = t_base0 + t2
                            if tile_st[t]:
                                agg_state[0] = pap.tile([128, 128], F32,
                                                        tag="psagg",
                                                        name="psagg")
                            nc.tensor.matmul(out=agg_state[0][:],
                                             lhsT=odt0[:, t2, :],
                                             rhs=contrib0[:, t2, :],
                                             start=bool(tile_st[t]),
                                             stop=bool(tile_sp[t]))
                            if tile_sp[t]:
                                sc = tile_sc[t]
                                nc.vector.tensor_add(accum[:, sc, :],
                                                     accum[:, sc, :],
                                                     agg_state[0][:])

                    prev = None
                    for b in range(NB):
                        g = tile_g[b * BT]
                        t_base = b * BT
                        gix = ep.tile([128, EB // 16], I16, tag="gix")
                        nc.sync.dma_start(
                            out=gix[:],
                            in_=din["gidx"][:, b * (EB // 16):(b + 1) * (EB // 16)])
                        od = ep.tile([128, EB], FP8, tag="od")
                        nc.sync.dma_start(out=od[:], in_=din["odh"][b, :, :])
                        odt = ep.tile([128, BT, 128], FP8, tag="odt")
                        nc.sync.dma_start(
                            out=odt[:],
                            in_=din["odth"][b, :, :].rearrange(
                                "p (t j) -> p t j", j=128))
                        srcr = ep.tile([128, BT, 128], BF16, tag="srcr")
                        # 512-idx bursts: stay under the SWDGE FIFO depth so
                        # descriptor generation never stalls on queue space
                        NQ = EB // 512
                        for hh in range(NQ):
                            nc.gpsimd.dma_gather(
                                out_ap=srcr[:, hh * (BT // NQ):(hh + 1) * (BT // NQ), :],
                                in_ap=fsh[g][:].rearrange("c s k -> (c s) k"),
                                idxs_ap=gix[:, hh * (EB // (16 * NQ)):(hh + 1) * (EB // (16 * NQ))],
                                num_idxs=EB // NQ, num_idxs_reg=EB // NQ,
                                elem_size=128,
                                single_packet=False,
                                queue_num=(NQ * b + hh) % 4)
                        # expand + alpha, pipelined at SUB granularity
                        SUB = cfg.SUB
                        contrib = ep.tile([128, BT, 128], BF16, tag="contrib")
                        for ww in range(BT // SUB):
                            ws = slice(ww * SUB, (ww + 1) * SUB)
                            psx = pxp.tile([128, SUB, HN], F32, tag="psx")
                            for t2 in range(SUB):
                                t = t_base + ww * SUB + t2
                                nc.tensor.matmul(
                                    out=psx[:, t2, :],
                                    lhsT=od[:, (ww * SUB + t2) * TE:(ww * SUB + t2 + 1) * TE],
                                    rhs=ad_nm[:, tile_sc[t], :],
                                    start=True, stop=True)
                            nc.vector.scalar_tensor_tensor(
                                out=contrib[:, ws, HN:128],
                                in0=psx[:], scalar=1.0,
                                in1=srcr[:, ws, 0:HN],
                                op0=AX.mult, op1=AX.add)
                            nc.scalar.activation(out=contrib[:, ws, HN:128],
                                                 in_=contrib[:, ws, HN:128],
                                                 func=AF.Prelu, alpha=cfg.NEG)
                            nc.scalar.activation(out=contrib[:, ws, HN:128],
                                                 in_=contrib[:, ws, HN:128],
                                                 func=AF.Exp)
                            nc.vector.tensor_mul(contrib[:, ws, 0:HN],
                                                 contrib[:, ws, HN:128],
                                                 srcr[:, ws, HN:128])
                        # aggregation for the PREVIOUS block — its contrib is
                        # ready, so the PE stream has no semaphore stalls
                        if prev is not None:
                            emit_aggs(prev)
                        prev = (t_base, odt, contrib)
                    emit_aggs(prev)

                # --- finalize ---
                with tc.tile_pool(name=f"fi{l}", bufs=2) as fp, \
                     tc.tile_pool(name=f"fp{l}", bufs=2, space="PSUM") as fpp:
                    den = fp.tile([128, NSC, HN], F32, tag="den")
                    nc.vector.tensor_scalar_add(den[:], accum[:, :, HN:128], 1e-16)
                    nc.vector.reciprocal(den[:], den[:])
                    if l < L - 1:
                        hnm = fp.tile([128, NSC, HN], BF16, tag="hnm")
                        nc.vector.tensor_mul(hnm[:], accum[:, :, 0:HN], den[:])
                        for sc in range(NSC):
                            c0 = sc * SC
                            n = min(SC, NL - c0)
                            pth = fpp.tile([HN, 128], BF16, tag="pth")
                            nc.tensor.transpose(out=pth[:], in_=hnm[:, sc, :],
                                                identity=ident[:, :])
                            nc.scalar.copy(out=hTb[:, c0:c0 + n], in_=pth[:, :n])
                    else:
                        hno = fp.tile([128, NSC, HN], F32, tag="hno")
                        nc.vector.tensor_mul(hno[:], accum[:, :, 0:HN], den[:])
                        nfull = (NSC - 1) * SC       # 12446
                        nc.sync.dma_start(
                            out=out[0:nfull, :].rearrange("(s d) k -> d s k", d=SC),
                            in_=hno[0:SC, 0:NSC - 1, :])
                        nc.sync.dma_start(
                            out=out[nfull:NL, :],
                            in_=hno[0:NL - nfull, NSC - 1, :])
    return nc


# --------------------------------------------------------------- entry ------
def kernel(**inputs):
    cfg = Cfg(BT=32)
    in_maps, meta = host_prepare(cfg, inputs)
    nc = build_nc(cfg, meta)
    nc.compile()
    split_sync_waits(nc)
    res = run_bass_kernel_spmd(nc, in_maps, core_ids=list(range(cfg.C)),
                               trace=bool(int(os.environ.get("GNN_TRACE", "0"))))
    if res.exec_time_ns is not None:
        print(f"HW exec time: {res.exec_time_ns} ns")
    h = np.concatenate([np.asarray(res.results[c]["out"])
                        for c in range(cfg.C)], axis=0)
    return h.reshape(cfg.B, -1, cfg.HN)



# revision 13
# speedup vs baseline: 1.0275x; 1.0275x over previous
"""GNN message-passing encoder (GAT-style) on 8 Trainium2 NeuronCores — v4.

Self-contained: hardcodes the problem shapes (N=100000, E=1600000, HN=64, L=3).

Design (edge-major, bf16, scatterless):
- Core c owns dst nodes [c*NL, (c+1)*NL). Node space re-keyed into G=4 chunks:
  chunk s = slice s of every core (SP=3200 padded rows per core-slice), so each
  chunk's table is completed by ONE sub-AllGather -> gather of chunk g overlaps
  the AllGather of chunk g+1.
- Per layer: table build emits node-major bf16 rows [a_s | hl] per slice,
  4 sub-AllGathers; a_d table kept local node-major (127-node subchunks,
  row 127 = u_l so the ea*u edge term rides the same expand matmul).
- Edges sorted by (chunk, dst-subchunk, dst); tiles of 128 edges never span a
  (chunk, subchunk); blocks of 32 tiles never span a chunk. Per block:
  dma_gather 4096 src rows (256B bf16); one-hot masks od (dst-major, via K=1
  broadcast matmul + is_eq) and odT (edge-major, via is_eq against iota);
  expand matmul a_d+ea*u per tile; alpha = psx + a_s; LeakyReLU; Exp;
  contrib = [ex*hl | ex]; aggregation matmul accumulates [128 dst, 128 ch]
  PSUM across the subchunk's tiles; flushed by DVE add into an SBUF
  accumulator — no scatter, no HBM accum round-trip.
- Finalize: h = num/(den+1e-16) from SBUF, PE-transpose back to channel-major
  for the next layer; final layer DMAs node-major rows out.
"""
import os
import sys

for _p in ("/opt/trn_rl_repo",):
    if _p not in sys.path:
        sys.path.insert(0, _p)

import numpy as np
import ml_dtypes

import concourse.bass as bass
import concourse.mybir as mybir
import concourse.tile as tile
from concourse import bacc
from concourse import library_config as libcfg
from concourse.bass_utils import run_bass_kernel_spmd

F32 = mybir.dt.float32
BF16 = mybir.dt.bfloat16
I16 = mybir.dt.int16
AX = mybir.AluOpType
AF = mybir.ActivationFunctionType
BF16NP = ml_dtypes.bfloat16
FP8 = mybir.dt.float8e4
ODT_NP = ml_dtypes.float8_e4m3


class Cfg:
    def __init__(self, N=100000, E=1600000, BT=32, B=10):
        self.N, self.E, self.C, self.HN, self.IN_N, self.L, self.B = \
            N, E, 8, 64, 3, 3, B
        self.NEG, self.EPS = 0.2, 1e-5
        self.NL = self.N // self.C          # 12500 local dst nodes
        self.G = 4                          # src chunks
        self.S = self.NL // self.G          # 3125 real rows per slice
        self.SP = -(-self.S // 128) * 128   # 3200 padded slice rows
        self.CH = self.C * self.SP          # 25600 chunk rows
        self.RP = self.SP // 128            # staging row-groups per slice
        self.SC = 127                       # dst nodes per subchunk
        self.NSC = -(-self.NL // self.SC)   # 99 subchunks
        self.TE = 128
        self.BT = BT
        self.SUB = min(8, BT)
        self.EB = self.TE * BT              # 4096 edges per block
        assert self.NL == self.G * self.S and self.CH <= 32768


# ------------------------------------------------------------- host prep ----
def _pack_idx16(vals):
    v = np.asarray(vals, dtype=np.int16)
    assert v.size % 16 == 0
    return np.tile(v.reshape(-1, 16).T, (8, 1)).copy()


def prep_edges(cfg, src, dst, ea):
    """Per-core edge partition grouped by (g, sc); returns per-core dict of
    (g, sc) -> (srcidx array, code array, ea array)."""
    per_core = []
    for c in range(cfg.C):
        m = (dst >= c * cfg.NL) & (dst < (c + 1) * cfg.NL)
        s_c, d_c, e_c = src[m], dst[m] - c * cfg.NL, ea[m]
        g = (s_c % cfg.NL) // cfg.S
        sidx = (s_c // cfg.NL) * cfg.SP + (s_c % cfg.S)
        sc = d_c // cfg.SC
        code = d_c % cfg.SC
        order = np.lexsort((d_c, sc, g))
        per_core.append(dict(g=g[order], sidx=sidx[order], sc=sc[order],
                             code=code[order], ea=e_c[order]))
    return per_core


def build_tiles(cfg, per_core):
    """Global tile layout (shared across cores) + per-core packed streams."""
    C, G, NSC, TE, BT = cfg.C, cfg.G, cfg.NSC, cfg.TE, cfg.BT
    # edge counts per (core, g, sc)
    cnt = np.zeros((C, G, NSC), np.int64)
    for c in range(C):
        pc = per_core[c]
        np.add.at(cnt[c], (pc["g"], pc["sc"]), 1)
    T = np.maximum(cnt, 0).max(axis=0)          # [G, NSC] max count
    T = -(-T // TE)                              # tiles per (g, sc)
    # pad each g to a multiple of BT (extend last subchunk's tile count)
    for g in range(G):
        rem = (-T[g].sum()) % BT
        T[g, NSC - 1] += rem
    NBg = [int(T[g].sum()) // BT for g in range(G)]
    TT = int(T.sum())
    NB = sum(NBg)
    assert TT == NB * BT

    # shared per-tile metadata
    tile_g = np.zeros(TT, np.int32)
    tile_sc = np.zeros(TT, np.int32)
    tile_st = np.zeros(TT, bool)
    tile_sp = np.zeros(TT, bool)
    t = 0
    for g in range(G):
        for sc in range(NSC):
            n = int(T[g, sc])
            if n == 0:
                continue
            tile_g[t:t + n] = g
            tile_sc[t:t + n] = sc
            tile_st[t] = True
            tile_sp[t + n - 1] = True
            t += n
    assert t == TT

    # per-core streams
    streams = []
    d127 = np.arange(127)
    for c in range(C):
        pc = per_core[c]
        gidx = np.zeros(TT * TE, np.int16)
        codeF = np.full(TT * TE, -1, np.int32)
        earow = np.zeros(TT * TE, np.float32)
        # group start offsets in the sorted stream
        t0 = 0
        p = 0  # pointer into sorted edge stream
        for g in range(G):
            for sc in range(NSC):
                n_edges = int(cnt[c, g, sc])
                ntiles = int(T[g, sc])
                base = t0 * TE
                sl = slice(p, p + n_edges)
                gidx[base:base + n_edges] = pc["sidx"][sl].astype(np.int16)
                codeF[base:base + n_edges] = pc["code"][sl]
                earow[base:base + n_edges] = pc["ea"][sl]
                p += n_edges
                t0 += ntiles
        assert p == len(pc["g"])
        # od[d, e] one-hot (dst-major) with ea on row 127; odT[p, t, d]
        od = np.zeros((128, TT * TE), ODT_NP)
        od[:127, :] = (codeF[None, :] == d127[:, None]).astype(ODT_NP)
        od[127, :] = earow.astype(ODT_NP)
        odT = (codeF.reshape(TT, TE).T[:, :, None]
               == d127[None, None, :]).astype(ODT_NP)   # [p, TT, 127]
        odTf = np.zeros((128, TT, 128), ODT_NP)
        odTf[:, :, :127] = odT
        streams.append(dict(
            gidx=_pack_idx16(gidx),
            odh=od.reshape(128, NB, cfg.EB).transpose(1, 0, 2).copy(),
            odth=odTf.reshape(128, NB, BT * 128).transpose(1, 0, 2).copy(),
        ))
    meta = dict(NB=NB, NBg=NBg, TT=TT, tile_g=tile_g, tile_sc=tile_sc,
                tile_st=tile_st, tile_sp=tile_sp)
    return meta, streams


def host_prepare(cfg, inputs):
    ei = np.asarray(inputs["edge_index"])
    src, dst = ei[0].astype(np.int64), ei[1].astype(np.int64)
    ea = np.asarray(inputs["edge_attr"], np.float32)[:, 0]
    per_core = prep_edges(cfg, src, dst, ea)
    meta, streams = build_tiles(cfg, per_core)

    HN, L = cfg.HN, cfg.L
    fnw = np.asarray(inputs["fc_node_w"], np.float32)
    fnb = np.asarray(inputs["fc_node_b"], np.float32)
    few = np.asarray(inputs["fc_edge_w"], np.float32)
    feb = np.asarray(inputs["fc_edge_b"], np.float32)
    gam = np.asarray(inputs["bn_gamma"], np.float32)
    bet = np.asarray(inputs["bn_beta"], np.float32)
    fcw = np.asarray(inputs["fc_w"], np.float32)
    fcb = np.asarray(inputs["fc_b"], np.float32)
    aw = np.asarray(inputs["attn_w"], np.float32)
    ab = np.asarray(inputs["attn_b"], np.float32)
    wdst, wsrc, we = aw[:, :HN, :], aw[:, HN:2 * HN, :], aw[:, 2 * HN:, :]
    u = np.stack([few[0] @ we[l] for l in range(L)])          # [L, 64]
    adb = np.stack([ab[l] + feb @ we[l] for l in range(L)])   # [L, 64]
    x = np.asarray(inputs["x"], np.float32)

    shared = dict(
        fnw=fnw, fnb=fnb.reshape(-1, 1), gam=gam.reshape(-1, 1),
        bet=bet.reshape(-1, 1),
        fcw=np.ascontiguousarray(fcw.transpose(1, 0, 2)).astype(BF16NP),
        wsrc=np.ascontiguousarray(wsrc.transpose(1, 0, 2)).astype(BF16NP),
        wdst=np.ascontiguousarray(wdst.transpose(1, 0, 2)).astype(BF16NP),
        fcb=np.ascontiguousarray(fcb.T), adb=np.ascontiguousarray(adb.T),
        urow=u.astype(BF16NP),                                 # [L, 64]
        urow_b=np.broadcast_to(u.astype(BF16NP)[:, None, :],
                               (L, cfg.NSC, HN)).copy(),
        identb=np.eye(128, dtype=BF16NP),
    )
    in_maps = []
    for c in range(cfg.C):
        im = dict(shared)
        im.update(streams[c])
        im["xT"] = np.ascontiguousarray(
            x[c * cfg.NL:(c + 1) * cfg.NL].T).astype(BF16NP)
        in_maps.append(im)
    return in_maps, meta


# --------------------------------------------------------------- builder ----
def split_sync_waits(nc, max_waits=1):
    for f in nc.m.functions:
        for bb in f.blocks:
            old = bb.instructions
            if not any(i.sync_info and i.sync_info.on_wait
                       and len(i.sync_info.on_wait) > max_waits for i in old):
                continue
            new = []
            for ins in old:
                si = ins.sync_info
                if si is not None and si.on_wait and len(si.on_wait) > max_waits:
                    waits = list(si.on_wait)
                    extra, keep = waits[:-max_waits], waits[-max_waits:]
                    for j, w in enumerate(extra):
                        nop = mybir.InstNoOp(name=f"{ins.name}-wc{j}", ins=[], outs=[])
                        nop.engine = ins.engine
                        nop.sync_info = mybir.SyncInfo(on_wait=[w], on_update=[])
                        new.append(nop)
                    si.on_wait = keep
                new.append(ins)
            bb.instructions = new
    return nc


def build_nc(cfg, meta):
    NL, G, S, SP, CH = cfg.NL, cfg.G, cfg.S, cfg.SP, cfg.CH
    SC, NSC, TE, BT, EB = cfg.SC, cfg.NSC, cfg.TE, cfg.BT, cfg.EB
    HN, L, C, N = cfg.HN, cfg.L, cfg.C, cfg.N
    NB, NBg, TT = meta["NB"], meta["NBg"], meta["TT"]
    tile_g, tile_sc = meta["tile_g"], meta["tile_sc"]
    tile_st, tile_sp = meta["tile_st"], meta["tile_sp"]
    RG = [list(range(C))]

    nc = bacc.Bacc(None, target_bir_lowering=False, num_swdge_queues=4)
    din = {}

    def ext(name, shape, dt=F32):
        din[name] = nc.dram_tensor(name, shape, dt, kind="ExternalInput")

    ext("xT", [cfg.IN_N, NL], BF16)
    ext("gidx", [128, TT * TE // 16], I16)
    ext("odh", [NB, 128, EB], FP8)
    ext("odth", [NB, 128, BT * 128], FP8)
    ext("fnw", [cfg.IN_N, HN]); ext("fnb", [HN, 1])
    ext("gam", [HN, 1]); ext("bet", [HN, 1])
    ext("fcw", [HN, L, HN], BF16); ext("fcb", [HN, L])
    ext("wsrc", [HN, L, HN], BF16); ext("wdst", [HN, L, HN], BF16)
    ext("adb", [HN, L])
    ext("urow", [L, HN], BF16)
    ext("urow_b", [L, NSC, HN], BF16)
    ext("identb", [128, 128], BF16)

    out = nc.dram_tensor("out", [NL, HN], F32, kind="ExternalOutput")
    lt = [nc.dram_tensor(f"lt{s}", [SP, 128], BF16) for s in range(G)]
    fsh = [nc.dram_tensor(f"fsh{s}", [C, SP, 128], BF16, addr_space="Shared")
           for s in range(G)]
    bn_loc = nc.dram_tensor("bn_loc", [HN, 2], F32)
    bn_sh = nc.dram_tensor("bn_sh", [HN, 2], F32, addr_space="Shared")

    with tile.TileContext(nc) as tc:
        with tc.tile_pool(name="persist", bufs=1) as pp:
            nc.gpsimd.load_library(libcfg.mlp)
            w = {}
            for nm, shp, dt in (
                ("fnw", [cfg.IN_N, HN], F32), ("fnb", [HN, 1], F32),
                ("gam", [HN, 1], F32), ("bet", [HN, 1], F32),
                ("fcw", [HN, L, HN], BF16), ("fcb", [HN, L], F32),
                ("wsrc", [HN, L, HN], BF16), ("wdst", [HN, L, HN], BF16),
                ("adb", [HN, L], F32), ("urow", [L, HN], BF16),
                ("identb", [128, 128], BF16),
            ):
                w[nm] = pp.tile(shp, dt, tag=nm, name=nm)
                nc.sync.dma_start(out=w[nm][:], in_=din[nm][:])
            ident = w["identb"]
            hTb = pp.tile([HN, NL], BF16, tag="hTb", name="hTb")
            ad_nm = pp.tile([128, NSC, HN], BF16, tag="ad_nm", name="ad_nm")
            accum = pp.tile([128, NSC, 128], F32, tag="accum", name="accum")

            # ---------------- input proj + BN (f32) ----------------
            with tc.tile_pool(name="bn", bufs=2) as bp, \
                 tc.tile_pool(name="bnsq", bufs=1) as bq, \
                 tc.tile_pool(name="bnp", bufs=2, space="PSUM") as bpp:
                xt = bq.tile([cfg.IN_N, NL], BF16, tag="xt")
                nc.sync.dma_start(out=xt[:], in_=din["xT"][:])
                fnwb = bp.tile([cfg.IN_N, HN], BF16, tag="fnwb")
                nc.vector.tensor_copy(fnwb[:], w["fnw"][:])
                for ci in range(-(-NL // 512)):
                    c0 = ci * 512
                    n = min(512, NL - c0)
                    ps = bpp.tile([HN, 512], F32, tag="ps")
                    nc.tensor.matmul(out=ps[:, :n], lhsT=fnwb[:],
                                     rhs=xt[:, c0:c0 + n], start=True, stop=True)
                    nc.vector.tensor_scalar_add(hTb[:, c0:c0 + n], ps[:, :n],
                                                w["fnb"][:])
                st = bp.tile([HN, 2], F32, tag="st")
                stp = bp.tile([HN, cfg.G], F32, tag="stp")
                sqc = bq.tile([HN, S], F32, tag="sqc")
                nc.vector.reduce_sum(st[:, 0:1], hTb[:], axis=mybir.AxisListType.X)
                for j in range(cfg.G):
                    nc.vector.scalar_tensor_tensor(
                        out=sqc[:], in0=hTb[:, j * S:(j + 1) * S], scalar=1.0,
                        in1=hTb[:, j * S:(j + 1) * S], op0=AX.mult, op1=AX.mult,
                        accum_out=stp[:, j:j + 1])
                nc.vector.reduce_sum(st[:, 1:2], stp[:], axis=mybir.AxisListType.X)
                nc.sync.dma_start(out=bn_loc[:], in_=st[:])
                nc.gpsimd.collective_compute("AllReduce", AX.add,
                                             replica_groups=RG,
                                             ins=[bn_loc[:]], outs=[bn_sh[:]])
                sg = bp.tile([HN, 2], F32, tag="sg")
                nc.sync.dma_start(out=sg[:], in_=bn_sh[:])
                mean = bp.tile([HN, 1], F32, tag="mean")
                var = bp.tile([HN, 1], F32, tag="var")
                nc.vector.tensor_scalar_mul(mean[:], sg[:, 0:1], 1.0 / N)
                nc.vector.tensor_scalar_mul(var[:], sg[:, 1:2], 1.0 / N)
                msq = bp.tile([HN, 1], F32, tag="msq")
                nc.vector.tensor_mul(msq[:], mean[:], mean[:])
                nc.vector.tensor_sub(var[:], var[:], msq[:])
                nc.vector.tensor_scalar_add(var[:], var[:], cfg.EPS)
                rs = bp.tile([HN, 1], F32, tag="rs")
                nc.scalar.activation(out=rs[:], in_=var[:], func=AF.Sqrt)
                nc.vector.reciprocal(rs[:], rs[:])
                scale = bp.tile([HN, 1], F32, tag="scale")
                nc.vector.tensor_mul(scale[:], rs[:], w["gam"][:])
                nbias = bp.tile([HN, 1], F32, tag="nbias")
                nc.vector.tensor_mul(nbias[:], mean[:], scale[:])
                nc.vector.scalar_tensor_tensor(out=nbias[:], in0=nbias[:],
                                               scalar=-1.0, in1=w["bet"][:],
                                               op0=AX.mult, op1=AX.add)
                nc.vector.tensor_scalar(out=hTb[:], in0=hTb[:], scalar1=scale[:],
                                        scalar2=nbias[:], op0=AX.mult, op1=AX.add)

            # ---------------- layers ----------------
            for l in range(L):
                # --- table build ---
                with tc.tile_pool(name=f"tb{l}", bufs=2) as tp, \
                     tc.tile_pool(name=f"tq{l}", bufs=1) as tq, \
                     tc.tile_pool(name=f"tp{l}", bufs=2, space="PSUM") as tpp:
                    adT = tq.tile([HN, NL], BF16, tag="adT")
                    nc.vector.memset(ad_nm[:], 0.0)
                    for s in range(G):
                        hls = tp.tile([HN, S], BF16, tag="hls")
                        asl = tp.tile([HN, S], BF16, tag="asl")
                        for ci in range(-(-S // 512)):
                            c0 = ci * 512
                            n = min(512, S - c0)
                            a0 = s * S + c0
                            ph = tpp.tile([HN, 512], F32, tag="ph")
                            nc.tensor.matmul(out=ph[:, :n], lhsT=w["fcw"][:, l, :],
                                             rhs=hTb[:, a0:a0 + n],
                                             start=True, stop=True)
                            nc.vector.tensor_scalar_add(hls[:, c0:c0 + n],
                                                        ph[:, :n],
                                                        w["fcb"][:, l:l + 1])
                            psa = tpp.tile([HN, 512], F32, tag="ph")
                            nc.tensor.matmul(out=psa[:, :n], lhsT=w["wsrc"][:, l, :],
                                             rhs=hls[:, c0:c0 + n],
                                             start=True, stop=True)
                            nc.scalar.copy(out=asl[:, c0:c0 + n], in_=psa[:, :n])
                            psd = tpp.tile([HN, 512], F32, tag="ph")
                            nc.tensor.matmul(out=psd[:, :n], lhsT=w["wdst"][:, l, :],
                                             rhs=hls[:, c0:c0 + n],
                                             start=True, stop=True)
                            nc.vector.tensor_scalar_add(adT[:, a0:a0 + n],
                                                        psd[:, :n],
                                                        w["adb"][:, l:l + 1])
                        # node-major rows for slice s
                        stg = tp.tile([128, cfg.RP, 128], BF16, tag="stg")
                        nc.vector.memset(stg[:], 0.0)
                        for r in range(cfg.RP):
                            c0 = r * 128
                            n = min(128, S - c0)
                            pt1 = tpp.tile([128, HN], BF16, tag="pt")
                            nc.tensor.transpose(out=pt1[:n, :],
                                                in_=asl[:, c0:c0 + n],
                                                identity=ident[:HN, :HN])
                            nc.scalar.copy(out=stg[:n, r, 0:HN], in_=pt1[:n, :])
                            pt2 = tpp.tile([128, HN], BF16, tag="pt")
                            nc.tensor.transpose(out=pt2[:n, :],
                                                in_=hls[:, c0:c0 + n],
                                                identity=ident[:HN, :HN])
                            nc.scalar.copy(out=stg[:n, r, HN:128], in_=pt2[:n, :])
                        nc.sync.dma_start(
                            out=lt[s][:, :].rearrange("(r p) k -> p r k", p=128),
                            in_=stg[:, :, :])
                        nc.gpsimd.collective_compute("AllGather", AX.bypass,
                                                     replica_groups=RG,
                                                     ins=[lt[s][:]],
                                                     outs=[fsh[s][:]])
                    # a_d node-major subtables
                    for sc in range(NSC):
                        c0 = sc * SC
                        n = min(SC, NL - c0)
                        pta = tpp.tile([128, HN], BF16, tag="pt")
                        nc.tensor.transpose(out=pta[:n, :],
                                            in_=adT[:, c0:c0 + n],
                                            identity=ident[:HN, :HN])
                        nc.scalar.copy(out=ad_nm[:n, sc, :], in_=pta[:n, :])
                    nc.sync.dma_start(out=ad_nm[127:128, :, :],
                                      in_=din["urow_b"][l:l + 1, :, :])
                    nc.vector.memset(accum[:], 0.0)

                # --- edge blocks ---
                with tc.tile_pool(name=f"ep{l}", bufs=3) as ep, \
                     tc.tile_pool(name=f"px{l}", bufs=4, space="PSUM") as pxp, \
                     tc.tile_pool(name=f"pa{l}", bufs=4, space="PSUM") as pap:
                    agg_state = [None]

                    def emit_aggs(prev):
                        t_base0, odt0, contrib0 = prev
                        for t2 in range(BT):
                            t = t_base0 + t2
                            if tile_st[t]:
                                agg_state[0] = pap.tile([128, 128], F32,
                                                        tag="psagg",
                                                        name="psagg")
                            nc.tensor.matmul(out=agg_state[0][:],
                                             lhsT=odt0[:, t2, :],
                                             rhs=contrib0[:, t2, :],
                                             start=bool(tile_st[t]),
                                             stop=bool(tile_sp[t]))
                            if tile_sp[t]:
                                sc = tile_sc[t]
                                nc.vector.tensor_add(accum[:, sc, :],
                                                     accum[:, sc, :],
                                                     agg_state[0][:])

                    prev = None
                    for b in range(NB):
                        g = tile_g[b * BT]
                        t_base = b * BT
                        gix = ep.tile([128, EB // 16], I16, tag="gix")
                        nc.scalar.dma_start(
                            out=gix[:],
                            in_=din["gidx"][:, b * (EB // 16):(b + 1) * (EB // 16)])
                        od = ep.tile([128, EB], FP8, tag="od")
                        nc.scalar.dma_start(out=od[:], in_=din["odh"][b, :, :])
                        odt = ep.tile([128, BT, 128], FP8, tag="odt")
                        nc.sync.dma_start(
                            out=odt[:],
                            in_=din["odth"][b, :, :].rearrange(
                                "p (t j) -> p t j", j=128))
                        srcr = ep.tile([128, BT, 128], BF16, tag="srcr")
                        NQ = 4 if BT % 4 == 0 else 1
                        for hh in range(NQ):
                            nc.gpsimd.dma_gather(
                                out_ap=srcr[:, hh * (BT // NQ):(hh + 1) * (BT // NQ), :],
                                in_ap=fsh[g][:].rearrange("c s k -> (c s) k"),
                                idxs_ap=gix[:, hh * (EB // (16 * NQ)):(hh + 1) * (EB // (16 * NQ))],
                                num_idxs=EB // NQ, num_idxs_reg=EB // NQ,
                                elem_size=128,
                                single_packet=False,
                                queue_num=(NQ * b + hh) % 4)
                        # expand + alpha, pipelined at SUB granularity
                        SUB = cfg.SUB
                        contrib = ep.tile([128, BT, 128], BF16, tag="contrib")
                        for ww in range(BT // SUB):
                            ws = slice(ww * SUB, (ww + 1) * SUB)
                            psx = pxp.tile([128, SUB, HN], F32, tag="psx")
                            for t2 in range(SUB):
                                t = t_base + ww * SUB + t2
                                nc.tensor.matmul(
                                    out=psx[:, t2, :],
                                    lhsT=od[:, (ww * SUB + t2) * TE:(ww * SUB + t2 + 1) * TE],
                                    rhs=ad_nm[:, tile_sc[t], :],
                                    start=True, stop=True)
                            nc.vector.scalar_tensor_tensor(
                                out=contrib[:, ws, HN:128],
                                in0=psx[:], scalar=1.0,
                                in1=srcr[:, ws, 0:HN],
                                op0=AX.mult, op1=AX.add)
                            nc.scalar.activation(out=contrib[:, ws, HN:128],
                                                 in_=contrib[:, ws, HN:128],
                                                 func=AF.Prelu, alpha=cfg.NEG)
                            nc.scalar.activation(out=contrib[:, ws, HN:128],
                                                 in_=contrib[:, ws, HN:128],
                                                 func=AF.Exp)
                            nc.vector.tensor_mul(contrib[:, ws, 0:HN],
                                                 contrib[:, ws, HN:128],
                                                 srcr[:, ws, HN:128])
                        # aggregation for the PREVIOUS block — its contrib is
                        # ready, so the PE stream has no semaphore stalls
                        if prev is not None:
                            emit_aggs(prev)
                        prev = (t_base, odt, contrib)
                    emit_aggs(prev)

                # --- finalize ---
                with tc.tile_pool(name=f"fi{l}", bufs=2) as fp, \
                     tc.tile_pool(name=f"fp{l}", bufs=2, space="PSUM") as fpp:
                    den = fp.tile([128, NSC, HN], F32, tag="den")
                    nc.vector.tensor_scalar_add(den[:], accum[:, :, HN:128], 1e-16)
                    nc.vector.reciprocal(den[:], den[:])
                    if l < L - 1:
                        hnm = fp.tile([128, NSC, HN], BF16, tag="hnm")
                        nc.vector.tensor_mul(hnm[:], accum[:, :, 0:HN], den[:])
                        for sc in range(NSC):
                            c0 = sc * SC
                            n = min(SC, NL - c0)
                            pth = fpp.tile([HN, 128], BF16, tag="pth")
                            nc.tensor.transpose(out=pth[:], in_=hnm[:, sc, :],
                                                identity=ident[:, :])
                            nc.scalar.copy(out=hTb[:, c0:c0 + n], in_=pth[:, :n])
                    else:
                        hno = fp.tile([128, NSC, HN], F32, tag="hno")
                        nc.vector.tensor_mul(hno[:], accum[:, :, 0:HN], den[:])
                        nfull = (NSC - 1) * SC       # 12446
                        nc.sync.dma_start(
                            out=out[0:nfull, :].rearrange("(s d) k -> d s k", d=SC),
                            in_=hno[0:SC, 0:NSC - 1, :])
                        nc.sync.dma_start(
                            out=out[nfull:NL, :],
                            in_=hno[0:NL - nfull, NSC - 1, :])
    return nc


# --------------------------------------------------------------- entry ------
def kernel(**inputs):
    cfg = Cfg(BT=32)
    in_maps, meta = host_prepare(cfg, inputs)
    nc = build_nc(cfg, meta)
    nc.compile()
    split_sync_waits(nc)
    res = run_bass_kernel_spmd(nc, in_maps, core_ids=list(range(cfg.C)),
                               trace=bool(int(os.environ.get("GNN_TRACE", "0"))))
    if res.exec_time_ns is not None:
        print(f"HW exec time: {res.exec_time_ns} ns")
    h = np.concatenate([np.asarray(res.results[c]["out"])
                        for c in range(cfg.C)], axis=0)
    return h.reshape(cfg.B, -1, cfg.HN)

